# revision 24
# baseline (speedup 1.0000x reference)
"""Trainium2 Bass kernel for nn_MILPFAttnTrexModel (segment_reduce).

Contract: kernel(**inputs) takes the FULL unsharded inputs (numpy arrays, keys
as in reference.setup_inputs()) and returns the FULL [G, NC] float32 output.

Strategy (8 NeuronCores, SPMD — one program, per-core data):
  - Host buckets rows by group; 8 groups per core, each group's tile-instance
    rows padded to a uniform block of TB columns. Tile-instance inputs are
    shipped feature-major AND pre-quantized to fp8 e4m3; the 2-layer MLP runs
    as fp8 DoubleRow matmuls (K=256 per pass, 0.5 cycles/row).
  - Algebraic fold: the whole v-projection + out_group matmul collapse into
    16 extra bf16 "u" columns next to the scores:
        out[g,l,c] = sum_i attn[i,l] * (v[i] . Wout[GL+l*LC:+LC, c])
                   = sum_i attn[i,l] * (xt2[i] . (Wv @ Wout_lc)) + bv.Wout_lc
    Scores columns are duplicated per class so ex[16,TB] and u[16,TB] align
    partition-wise; one fused DVE multiply+row-reduce gives the result.
    bk and bv contributions are softmax-invariant / constant and folded on
    host.  Pad columns are killed exactly by an extra K-row in the scores
    matmul contributing -1e30 * pad_flag.
  - Whole-image branch (1 instance per group): transposed bf16 MLP with the
    bias folded in as an extra contraction row; [Wt, GL] output, segment max
    on host.
"""

import math
import os
import numpy as np
import ml_dtypes

import concourse.bacc as bacc
import concourse.tile as tile
from concourse import mybir
from concourse.bass_utils import run_bass_kernel_spmd
from concourse.masks import make_identity

# Set by the most recent kernel() call when KERNEL_TRACE=1 (dev-only).
last_exec_time_ns = None
last_mean_exec_time_ns = None


def _install_ntff_shim():
    """Register the axon NTFF profile hook if the image's antenv lacks it."""
    import sys, types
    try:
        import antenv.axon_hooks  # noqa: F401
        return
    except ImportError:
        pass
    m = types.ModuleType("antenv.axon_hooks")
    m._hook = None
    m.set_axon_ntff_profile_hook = lambda h: setattr(m, "_hook", h)
    m.get_axon_ntff_profile_hook = lambda: m._hook
    sys.modules["antenv.axon_hooks"] = m
    import antenv
    antenv.axon_hooks = m
    from trn_agent_boot.trn_boot import _ntff_profile_via_ctypes
    m.set_axon_ntff_profile_hook(
        _ntff_profile_via_ctypes("/opt/axon/libaxon_pjrt.so"))

F32 = mybir.dt.float32
BF16 = mybir.dt.bfloat16
FP8 = mybir.dt.float8e4
AX = mybir.AxisListType
ALU = mybir.AluOpType
ACTF = mybir.ActivationFunctionType
DR = mybir.MatmulPerfMode.DoubleRow

NP_FP8 = ml_dtypes.float8_e4m3
NP_BF16 = ml_dtypes.bfloat16

N_CORES = 8
G = 64
GPC = G // N_CORES          # groups per core
IN = 1024
GL = 512
LC = 256
L = 8
NCLS = 2
NSU = 2 * L * NCLS          # 32 = 16 dup-score cols + 16 u cols
NEGBIG = -1.0e30

_prog_cache = {}


def _ceil_to(x, m):
    return ((x + m - 1) // m) * m


def _build_program(TB, WB):
    """Build the SPMD Tile program for block sizes (TB, WB)."""
    T = GPC * TB
    Wt = GPC * WB
    NSZ = 384
    NCH = TB // NSZ          # N-chunks per group
    KW = IN // 128 + 1       # whole-branch K chunks incl. bias-aug row

    nc = bacc.Bacc("TRN2", target_bir_lowering=False, debug=False,
                   num_devices=N_CORES)

    xtq = nc.dram_tensor("xtq", [GPC * 128, IN // 128, TB], FP8,
                         kind="ExternalInput")
    padf = nc.dram_tensor("padf", [1, T], BF16, kind="ExternalInput")
    wl0q = nc.dram_tensor("wl0q", [128, IN // 128, GL], FP8,
                          kind="ExternalInput")
    wl1q = nc.dram_tensor("wl1q", [128, GL // 128, LC], FP8,
                          kind="ExternalInput")
    scw = nc.dram_tensor("scw", [128, LC // 128, NSU], BF16,
                         kind="ExternalInput")
    bl0t = nc.dram_tensor("bl0t", [128, GL // 128], F32, kind="ExternalInput")
    bl1t = nc.dram_tensor("bl1t", [128, LC // 128], F32, kind="ExternalInput")
    negbig_in = nc.dram_tensor("negbig", [1, NSU], BF16, kind="ExternalInput")
    xwt = nc.dram_tensor("xwt", [128, KW, Wt], BF16, kind="ExternalInput")
    wg0b = nc.dram_tensor("wg0b", [128, KW, 2 * GL], BF16,
                          kind="ExternalInput")
    wg1b = nc.dram_tensor("wg1b", [128, KW, GL], BF16, kind="ExternalInput")
    out16 = nc.dram_tensor("out16", [2 * L * NCLS // 2, GPC], F32,
                           kind="ExternalOutput")
    out_w = nc.dram_tensor("out_w", [Wt, GL], F32, kind="ExternalOutput")

    tick = [0]

    def evac(out_ap, in_ap, bias_ap=None, force=None):
        """PSUM -> SBUF eviction, optionally fused bias-add + relu."""
        use_dve = (tick[0] % 2 == 0) if force is None else (force == "dve")
        tick[0] += 1
        if bias_ap is None:
            if use_dve:
                nc.vector.tensor_copy(out_ap, in_ap)
            else:
                nc.scalar.copy(out_ap, in_ap)
        else:
            if use_dve:
                nc.vector.tensor_scalar(out_ap, in_ap, bias_ap, 0.0,
                                        op0=ALU.add, op1=ALU.max)
            else:
                nc.scalar.activation(out_ap, in_ap, ACTF.Relu, bias=bias_ap)

    def emit_whole(nc, wgpool, wtpool, pw, pt, wg0_sb, wg1_sb, xw_sb, identb):
        # L1w: h1wT [Wt, 2GL] = relu(xw.T @ Wg0 + bg0)  (bias via aug K-row)
        h1w_sb = wtpool.tile([Wt, 2 * GL], BF16)
        for nchk in range(2 * GL // 512):
            ps = pw.tile([Wt, 512], F32, tag="pw")
            for kt in range(KW):
                nc.tensor.matmul(
                    ps, xw_sb[:, kt, :],
                    wg0_sb[:, kt, nchk * 512:(nchk + 1) * 512],
                    start=(kt == 0), stop=(kt == KW - 1))
            nc.scalar.activation(h1w_sb[:, nchk * 512:(nchk + 1) * 512],
                                 ps, ACTF.Relu)

        # transpose h1wT -> [128, KW, Wt] (+ bias-aug plane)
        h1t_sb = wtpool.tile([128, KW, Wt], BF16)
        nc.vector.memset(h1t_sb[:, KW - 1, :], 0.0)
        nc.vector.memset(h1t_sb[0:1, KW - 1, :], 1.0)
        for c in range(2 * GL // 128):
            pst = pt.tile([128, Wt], BF16, tag="pt")
            nc.tensor.transpose(pst, h1w_sb[:, c * 128:(c + 1) * 128],
                                identb[0:Wt, 0:Wt])
            nc.scalar.copy(h1t_sb[:, c, :], pst)

        # L2w: h2wT [Wt, GL] = relu(h1w.T @ Wg1 + bg1) -> host max
        h2w_sb = wtpool.tile([Wt, GL], F32)
        ps = pw.tile([Wt, 512], F32, tag="pw")
        for kt in range(KW):
            nc.tensor.matmul(ps, h1t_sb[:, kt, :], wg1_sb[:, kt, :],
                             start=(kt == 0), stop=(kt == KW - 1))
        nc.scalar.activation(h2w_sb, ps, ACTF.Relu)
        nc.gpsimd.dma_start(out=out_w.ap(), in_=h2w_sb)

    with tile.TileContext(nc) as tc:
        with (
            tc.tile_pool(name="weights", bufs=1) as wpool,
            tc.tile_pool(name="xt", bufs=2) as xtpool,
            tc.tile_pool(name="h1", bufs=2) as h1pool,
            tc.tile_pool(name="xt2", bufs=2) as xt2pool,
            tc.tile_pool(name="su", bufs=2) as supool,
            tc.tile_pool(name="small", bufs=2) as smpool,
            tc.tile_pool(name="ogall", bufs=1) as ogpool,
            tc.tile_pool(name="wg", bufs=1) as wgpool,
            tc.tile_pool(name="wtile", bufs=1) as wtpool,
            tc.tile_pool(name="ph1", bufs=3, space="PSUM") as ph1,
            tc.tile_pool(name="psc", bufs=2, space="PSUM") as psc,
            tc.tile_pool(name="pscu", bufs=1, space="PSUM") as pscu,
            tc.tile_pool(name="pw", bufs=1, space="PSUM") as pw,
            tc.tile_pool(name="pt", bufs=1, space="PSUM") as pt,
        ):
            wl0_sb = wpool.tile([128, IN // 128, GL], FP8)
            nc.gpsimd.dma_start(out=wl0_sb, in_=wl0q.ap())
            bl0_sb = wpool.tile([128, GL // 128], F32)
            nc.gpsimd.dma_start(out=bl0_sb, in_=bl0t.ap())
            wl1_sb = wpool.tile([128, GL // 128, LC], FP8)
            nc.gpsimd.dma_start(out=wl1_sb, in_=wl1q.ap())
            bl1_sb = wpool.tile([128, LC // 128], F32)
            nc.gpsimd.dma_start(out=bl1_sb, in_=bl1t.ap())
            scw_sb = wpool.tile([128, LC // 128, NSU], BF16)
            nc.gpsimd.dma_start(out=scw_sb, in_=scw.ap())
            negbig_sb = wpool.tile([1, NSU], BF16)
            nc.gpsimd.dma_start(out=negbig_sb, in_=negbig_in.ap())

            # whole-branch loads start immediately on the gpsimd queue and
            # overlap tile-branch compute
            wg0_sb = wgpool.tile([128, KW, 2 * GL], BF16)
            nc.gpsimd.dma_start(out=wg0_sb, in_=wg0b.ap())
            wg1_sb = wgpool.tile([128, KW, GL], BF16)
            nc.gpsimd.dma_start(out=wg1_sb, in_=wg1b.ap())
            xw_sb = wgpool.tile([128, KW, Wt], BF16)
            nc.gpsimd.dma_start(out=xw_sb, in_=xwt.ap())
            identb = wgpool.tile([128, 128], BF16)
            make_identity(nc, identb)

            og_sb = ogpool.tile([2 * L, GPC], F32)
            xtq_r = xtq.ap()

            for j in range(GPC):
                if j == 2:
                    # emit whole branch mid-stream: overlaps groups 2..7
                    emit_whole(nc, wgpool, wtpool, pw, pt,
                               wg0_sb, wg1_sb, xw_sb, identb)
                c0 = j * TB
                xt_sb = xtpool.tile([128, IN // 128, TB], FP8)
                nc.sync.dma_start(out=xt_sb,
                                  in_=xtq_r[j * 128:(j + 1) * 128])
                pf_sb = smpool.tile([1, TB], BF16, tag="pf")
                nc.scalar.dma_start(out=pf_sb,
                                    in_=padf.ap()[0:1, c0:c0 + TB])

                # L1: h1 = relu(Wl0.T @ xt + bl0)   [512, TB] fp8
                h1_sb = h1pool.tile([128, GL // 128, TB], FP8)
                for mc in range(GL // 128):
                    for s in range(NCH):
                        n0 = s * NSZ
                        ps = ph1.tile([128, NSZ], F32, tag="ps")
                        for k2 in range(IN // 256):
                            nc.tensor.matmul(
                                ps,
                                wl0_sb[:, 2 * k2:2 * k2 + 2,
                                       mc * 128:(mc + 1) * 128],
                                xt_sb[:, 2 * k2:2 * k2 + 2, n0:n0 + NSZ],
                                start=(k2 == 0), stop=(k2 == IN // 256 - 1),
                                perf_mode=DR)
                        nc.scalar.activation(h1_sb[:, mc, n0:n0 + NSZ], ps,
                                             ACTF.Relu,
                                             bias=bl0_sb[:, mc:mc + 1])

                # L2: xt2 = relu(Wl1.T @ h1 + bl1)  [256, TB] bf16
                xt2_sb = xt2pool.tile([128, LC // 128, TB], BF16)
                for mc in range(LC // 128):
                    for s in range(NCH):
                        n0 = s * NSZ
                        ps = ph1.tile([128, NSZ], F32, tag="ps")
                        for k2 in range(GL // 256):
                            nc.tensor.matmul(
                                ps,
                                wl1_sb[:, 2 * k2:2 * k2 + 2,
                                       mc * 128:(mc + 1) * 128],
                                h1_sb[:, 2 * k2:2 * k2 + 2, n0:n0 + NSZ],
                                start=(k2 == 0), stop=(k2 == GL // 256 - 1),
                                perf_mode=DR)
                        evac(xt2_sb[:, mc, n0:n0 + NSZ], ps,
                             bl1_sb[:, mc:mc + 1])

                # scores(dup) [16, TB] and u [16, TB]: separate PSUM groups
                # so every engine AP stays at partition base 0.
                sc_sb = supool.tile([2 * L, TB], F32, tag="sc")
                u_sb = supool.tile([2 * L, TB], F32, tag="u")
                for s in range(NCH):
                    n0 = s * NSZ
                    ps = psc.tile([2 * L, NSZ], F32, tag="psc")
                    psu = pscu.tile([2 * L, NSZ], F32, tag="psu")
                    for kt in range(LC // 128):
                        nc.tensor.matmul(
                            ps, scw_sb[:, kt, 0:2 * L],
                            xt2_sb[:, kt, n0:n0 + NSZ],
                            start=(kt == 0), stop=False)
                    nc.tensor.matmul(ps, negbig_sb[0:1, 0:2 * L],
                                     pf_sb[0:1, n0:n0 + NSZ],
                                     start=False, stop=True)
                    for kt in range(LC // 128):
                        nc.tensor.matmul(
                            psu, scw_sb[:, kt, 2 * L:NSU],
                            xt2_sb[:, kt, n0:n0 + NSZ],
                            start=(kt == 0), stop=(kt == LC // 128 - 1))
                    nc.vector.tensor_copy(sc_sb[:, n0:n0 + NSZ], ps)
                    nc.vector.tensor_copy(u_sb[:, n0:n0 + NSZ], psu)

                # segment softmax + fused u-reduction.  Scores are O(1)
                # (bounded inputs), so exp() needs no max-shift: softmax is
                # shift-invariant and fp32 exp cannot overflow here.
                ex_sb = supool.tile([2 * L, TB], F32, tag="ex")
                den = smpool.tile([2 * L, 1], F32, tag="den")
                nc.scalar.activation(ex_sb, sc_sb, ACTF.Exp,
                                     bias=0.0, accum_out=den)
                prod_sb = supool.tile([2 * L, TB], F32, tag="prod")
                red = smpool.tile([2 * L, 1], F32, tag="red")
                nc.vector.scalar_tensor_tensor(
                    prod_sb, ex_sb, 1.0, u_sb,
                    op0=ALU.mult, op1=ALU.mult, accum_out=red)
                rden = smpool.tile([2 * L, 1], F32, tag="rden")
                nc.vector.reciprocal(rden, den)
                nc.vector.tensor_scalar_mul(og_sb[:, j:j + 1], red, rden)

            nc.sync.dma_start(out=out16.ap(), in_=og_sb)

    nc.compile()
    return nc


def _get_program(key):
    if key not in _prog_cache:
        _prog_cache[key] = _build_program(*key)
    return _prog_cache[key]


def kernel(**inputs):
    x = np.ascontiguousarray(np.asarray(inputs["x"], dtype=np.float32))
    group = np.asarray(inputs["group"]).astype(np.int64)
    itype = np.asarray(inputs["instance_type"]).astype(np.int64)
    Wl0 = np.asarray(inputs["Wl0"], np.float32)
    bl0 = np.asarray(inputs["bl0"], np.float32)
    Wl1 = np.asarray(inputs["Wl1"], np.float32)
    bl1 = np.asarray(inputs["bl1"], np.float32)
    Wg0 = np.asarray(inputs["Wg0"], np.float32)
    bg0 = np.asarray(inputs["bg0"], np.float32)
    Wg1 = np.asarray(inputs["Wg1"], np.float32)
    bg1 = np.asarray(inputs["bg1"], np.float32)
    Wk = np.asarray(inputs["Wk"], np.float32)
    bk = np.asarray(inputs["bk"], np.float32)
    Wv = np.asarray(inputs["Wv"], np.float32)
    bv = np.asarray(inputs["bv"], np.float32)
    latent = np.asarray(inputs["latent"], np.float32)
    Wout = np.asarray(inputs["Wout"], np.float32)
    bout = np.asarray(inputs["bout"], np.float32)

    # ---- host bucketing ----
    is_tile = itype == 1
    is_whole = itype == 0
    tile_idx = [np.where(is_tile & (group == g))[0] for g in range(G)]
    whole_idx = [np.where(is_whole & (group == g))[0] for g in range(G)]
    ng = np.array([len(ix) for ix in tile_idx])
    wg = np.array([len(ix) for ix in whole_idx])
    TB = max(384, _ceil_to(int(ng.max()), 384))
    WB = max(1, int(wg.max()))
    T = GPC * TB
    Wt = GPC * WB
    KW = IN // 128 + 1

    # ---- shared staged weights ----
    scale = 1.0 / math.sqrt(LC)
    wkl = (Wk @ latent.T) * scale                     # [LC, L]
    wvo = np.stack([Wv @ Wout[GL + l * LC:GL + (l + 1) * LC, c]
                    for l in range(L) for c in range(NCLS)], axis=1)  # [LC,16]
    scw_host = np.concatenate(
        [np.repeat(wkl, NCLS, axis=1), wvo], axis=1)  # [LC, 32]
    negbig_host = np.zeros((1, NSU), np.float32)
    negbig_host[0, :2 * L] = NEGBIG

    wg0b = np.zeros((KW * 128, 2 * GL), np.float32)
    wg0b[:IN] = Wg0
    wg0b[IN] = bg0
    wg1b = np.zeros((KW * 128, GL), np.float32)
    wg1b[:IN] = Wg1
    wg1b[IN] = bg1

    def tile_km(a, np_dt):
        """[K, M] -> [128, K//128, M] (SBUF partition-major tiling)."""
        k, m = a.shape
        return np.ascontiguousarray(
            np.asarray(a, np.float32).astype(np_dt)
            .reshape(k // 128, 128, m).transpose(1, 0, 2))

    shared = dict(
        wl0q=tile_km(Wl0, NP_FP8), wl1q=tile_km(Wl1, NP_FP8),
        scw=tile_km(scw_host, NP_BF16),
        negbig=negbig_host.astype(NP_BF16),
        bl0t=np.ascontiguousarray(bl0.reshape(-1, 128).T),
        bl1t=np.ascontiguousarray(bl1.reshape(-1, 128).T),
        wg0b=tile_km(wg0b, NP_BF16), wg1b=tile_km(wg1b, NP_BF16),
    )

    xq_all = x.astype(NP_FP8)
    xb_all = x.astype(NP_BF16)

    in_maps = []
    for c in range(N_CORES):
        xtq = np.zeros((GPC * 128, IN // 128, TB), NP_FP8)
        xwt = np.zeros((KW * 128, Wt), np.float32)
        xwt[IN] = 1.0
        padf = np.ones((1, T), NP_BF16)
        for j in range(GPC):
            g = c * GPC + j
            ti, wi = tile_idx[g], whole_idx[g]
            n = len(ti)
            xtq[j * 128:(j + 1) * 128, :, :n] = (
                xq_all[ti].T.reshape(IN // 128, 128, n).transpose(1, 0, 2))
            xwt[:IN, j * WB:j * WB + len(wi)] = x[wi].T
            padf[0, j * TB:j * TB + n] = 0
        in_maps.append(dict(xtq=xtq, xwt=tile_km(xwt, NP_BF16), padf=padf,
                            **shared))

    nc = _get_program((TB, WB))
    trace = os.environ.get("KERNEL_TRACE") == "1"
    if trace:
        _install_ntff_shim()
    res = run_bass_kernel_spmd(nc, in_maps, core_ids=list(range(N_CORES)),
                               trace=trace)
    global last_exec_time_ns, last_mean_exec_time_ns
    last_exec_time_ns = res.exec_time_ns
    last_mean_exec_time_ns = res.mean_exec_time_ns

    # ---- host assembly ----
    whole_agg = np.empty((G, GL), np.float32)
    t2 = np.empty((G, NCLS), np.float32)
    for c in range(N_CORES):
        ow = res.results[c]["out_w"]          # [Wt, GL]
        og = res.results[c]["out16"]          # [16, GPC]
        for j in range(GPC):
            g = c * GPC + j
            if wg[g] == 0:
                whole_agg[g] = -np.inf
            else:
                whole_agg[g] = ow[j * WB:j * WB + wg[g]].max(axis=0)
            t2[g] = og[:, j].reshape(L, NCLS).sum(axis=0)
    const = np.array([sum(bv @ Wout[GL + l * LC:GL + (l + 1) * LC, c]
                          for l in range(L)) for c in range(NCLS)], np.float32)
    return (whole_agg @ Wout[:GL] + t2 + const + bout).astype(np.float32)


# revision 25
# speedup vs baseline: 1.3002x; 1.3002x over previous
"""Trainium2 Bass kernel for nn_MILPFAttnTrexModel (segment_reduce).

Contract: kernel(**inputs) takes the FULL unsharded inputs (numpy arrays, keys
as in reference.setup_inputs()) and returns the FULL [G, NC] float32 output.

Strategy (8 NeuronCores, SPMD — one program, per-core data):
  - Host buckets rows by group; 8 groups per core, each group's tile-instance
    rows padded to a uniform block of TB columns. Tile-instance inputs are
    shipped feature-major AND pre-quantized to fp8 e4m3; the 2-layer MLP runs
    as fp8 DoubleRow matmuls (K=256 per pass, 0.5 cycles/row).
  - Algebraic fold: the whole v-projection + out_group matmul collapse into
    16 extra bf16 "u" columns next to the scores:
        out[g,l,c] = sum_i attn[i,l] * (v[i] . Wout[GL+l*LC:+LC, c])
                   = sum_i attn[i,l] * (xt2[i] . (Wv @ Wout_lc)) + bv.Wout_lc
    Scores columns are duplicated per class so ex[16,TB] and u[16,TB] align
    partition-wise; one fused DVE multiply+row-reduce gives the result.
    bk and bv contributions are softmax-invariant / constant and folded on
    host.  Pad columns are killed exactly by an extra K-row in the scores
    matmul contributing -1e30 * pad_flag.
  - Whole-image branch (1 instance per group): transposed bf16 MLP with the
    bias folded in as an extra contraction row; [Wt, GL] output, segment max
    on host.
"""

import math
import os
import numpy as np
import ml_dtypes

import concourse.bacc as bacc
import concourse.tile as tile
from concourse import mybir
from concourse.bass_utils import run_bass_kernel_spmd
from concourse.masks import make_identity

# Set by the most recent kernel() call when KERNEL_TRACE=1 (dev-only).
last_exec_time_ns = None
last_mean_exec_time_ns = None


def _install_ntff_shim():
    """Register the axon NTFF profile hook if the image's antenv lacks it."""
    import sys, types
    try:
        import antenv.axon_hooks  # noqa: F401
        return
    except ImportError:
        pass
    m = types.ModuleType("antenv.axon_hooks")
    m._hook = None
    m.set_axon_ntff_profile_hook = lambda h: setattr(m, "_hook", h)
    m.get_axon_ntff_profile_hook = lambda: m._hook
    sys.modules["antenv.axon_hooks"] = m
    import antenv
    antenv.axon_hooks = m
    from trn_agent_boot.trn_boot import _ntff_profile_via_ctypes
    m.set_axon_ntff_profile_hook(
        _ntff_profile_via_ctypes("/opt/axon/libaxon_pjrt.so"))

F32 = mybir.dt.float32
BF16 = mybir.dt.bfloat16
FP8 = mybir.dt.float8e4
AX = mybir.AxisListType
ALU = mybir.AluOpType
ACTF = mybir.ActivationFunctionType
DR = mybir.MatmulPerfMode.DoubleRow

NP_FP8 = ml_dtypes.float8_e4m3
NP_BF16 = ml_dtypes.bfloat16

N_CORES = 8
G = 64
GPC = G // N_CORES          # groups per core
IN = 1024
GL = 512
LC = 256
L = 8
NCLS = 2
NSU = 2 * L * NCLS          # 32 = 16 dup-score cols + 16 u cols
NEGBIG = -1.0e30

_prog_cache = {}


def _ceil_to(x, m):
    return ((x + m - 1) // m) * m


def _build_program(TB, WB):
    """Build the SPMD Tile program for block sizes (TB, WB)."""
    T = GPC * TB
    Wt = GPC * WB
    NSZ = 384
    NCH = TB // NSZ          # N-chunks per group
    KW = IN // 128 + 1       # whole-branch K chunks incl. bias-aug row

    nc = bacc.Bacc("TRN2", target_bir_lowering=False, debug=False,
                   num_devices=N_CORES)

    xtq = nc.dram_tensor("xtq", [GPC * 128, IN // 128, TB], FP8,
                         kind="ExternalInput")
    padf = nc.dram_tensor("padf", [1, T], BF16, kind="ExternalInput")
    wl0q = nc.dram_tensor("wl0q", [128, IN // 128, GL], FP8,
                          kind="ExternalInput")
    wl1q = nc.dram_tensor("wl1q", [128, GL // 128, LC], FP8,
                          kind="ExternalInput")
    scw = nc.dram_tensor("scw", [128, LC // 128, NSU], BF16,
                         kind="ExternalInput")
    bl0t = nc.dram_tensor("bl0t", [128, GL // 128], F32, kind="ExternalInput")
    bl1t = nc.dram_tensor("bl1t", [128, LC // 128], F32, kind="ExternalInput")
    negbig_in = nc.dram_tensor("negbig", [1, NSU], BF16, kind="ExternalInput")
    xwt = nc.dram_tensor("xwt", [128, KW, Wt], BF16, kind="ExternalInput")
    wg0b = nc.dram_tensor("wg0b", [128, KW, 2 * GL], BF16,
                          kind="ExternalInput")
    wg1b = nc.dram_tensor("wg1b", [128, KW, GL], BF16, kind="ExternalInput")
    out16 = nc.dram_tensor("out16", [2 * L * NCLS // 2, GPC], F32,
                           kind="ExternalOutput")
    out_w = nc.dram_tensor("out_w", [Wt, GL], F32, kind="ExternalOutput")

    tick = [0]

    def evac(out_ap, in_ap, bias_ap=None, force=None):
        """PSUM -> SBUF eviction, optionally fused bias-add + relu."""
        use_dve = (tick[0] % 2 == 0) if force is None else (force == "dve")
        tick[0] += 1
        if bias_ap is None:
            if use_dve:
                nc.vector.tensor_copy(out_ap, in_ap)
            else:
                nc.scalar.copy(out_ap, in_ap)
        else:
            if use_dve:
                nc.vector.tensor_scalar(out_ap, in_ap, bias_ap, 0.0,
                                        op0=ALU.add, op1=ALU.max)
            else:
                nc.scalar.activation(out_ap, in_ap, ACTF.Relu, bias=bias_ap)

    def emit_whole(nc, wgpool, wtpool, pw, pt, wg0_sb, wg1_sb, xw_sb, identb):
        # L1w: h1wT [Wt, 2GL] = relu(xw.T @ Wg0 + bg0)  (bias via aug K-row)
        h1w_sb = wtpool.tile([Wt, 2 * GL], BF16)
        for nchk in range(2 * GL // 512):
            ps = pw.tile([Wt, 512], F32, tag="pw")
            for kt in range(KW):
                nc.tensor.matmul(
                    ps, xw_sb[:, kt, :],
                    wg0_sb[:, kt, nchk * 512:(nchk + 1) * 512],
                    start=(kt == 0), stop=(kt == KW - 1))
            nc.scalar.activation(h1w_sb[:, nchk * 512:(nchk + 1) * 512],
                                 ps, ACTF.Relu)

        # transpose h1wT -> [128, KW, Wt] (+ bias-aug plane)
        h1t_sb = wtpool.tile([128, KW, Wt], BF16)
        nc.vector.memset(h1t_sb[:, KW - 1, :], 0.0)
        nc.vector.memset(h1t_sb[0:1, KW - 1, :], 1.0)
        for c in range(2 * GL // 128):
            pst = pt.tile([128, Wt], BF16, tag="pt")
            nc.tensor.transpose(pst, h1w_sb[:, c * 128:(c + 1) * 128],
                                identb[0:Wt, 0:Wt])
            nc.scalar.copy(h1t_sb[:, c, :], pst)

        # L2w: h2wT [Wt, GL] = relu(h1w.T @ Wg1 + bg1) -> host max
        h2w_sb = wtpool.tile([Wt, GL], F32)
        ps = pw.tile([Wt, 512], F32, tag="pw")
        for kt in range(KW):
            nc.tensor.matmul(ps, h1t_sb[:, kt, :], wg1_sb[:, kt, :],
                             start=(kt == 0), stop=(kt == KW - 1))
        nc.scalar.activation(h2w_sb, ps, ACTF.Relu)
        nc.gpsimd.dma_start(out=out_w.ap(), in_=h2w_sb)

    with tile.TileContext(nc) as tc:
        with (
            tc.tile_pool(name="weights", bufs=1) as wpool,
            tc.tile_pool(name="xt", bufs=2) as xtpool,
            tc.tile_pool(name="h1", bufs=2) as h1pool,
            tc.tile_pool(name="xt2", bufs=2) as xt2pool,
            tc.tile_pool(name="su", bufs=2) as supool,
            tc.tile_pool(name="small", bufs=2) as smpool,
            tc.tile_pool(name="ogall", bufs=1) as ogpool,
            tc.tile_pool(name="wg", bufs=1) as wgpool,
            tc.tile_pool(name="wtile", bufs=1) as wtpool,
            tc.tile_pool(name="ph1", bufs=4, space="PSUM") as ph1,
            tc.tile_pool(name="psc", bufs=2, space="PSUM") as psc,
            tc.tile_pool(name="pw", bufs=1, space="PSUM") as pw,
            tc.tile_pool(name="pt", bufs=1, space="PSUM") as pt,
        ):
            wl0_sb = wpool.tile([128, IN // 128, GL], FP8)
            nc.gpsimd.dma_start(out=wl0_sb, in_=wl0q.ap())
            bl0_sb = wpool.tile([128, GL // 128], F32)
            nc.gpsimd.dma_start(out=bl0_sb, in_=bl0t.ap())
            wl1_sb = wpool.tile([128, GL // 128, LC], FP8)
            nc.gpsimd.dma_start(out=wl1_sb, in_=wl1q.ap())
            bl1_sb = wpool.tile([128, LC // 128], F32)
            nc.gpsimd.dma_start(out=bl1_sb, in_=bl1t.ap())
            scw_sb = wpool.tile([128, LC // 128, NSU], BF16)
            nc.gpsimd.dma_start(out=scw_sb, in_=scw.ap())
            negbig_sb = wpool.tile([1, NSU], BF16)
            nc.gpsimd.dma_start(out=negbig_sb, in_=negbig_in.ap())
            pfall_sb = wpool.tile([1, T], BF16)
            nc.scalar.dma_start(out=pfall_sb, in_=padf.ap())

            # whole-branch loads start immediately on the gpsimd queue and
            # overlap tile-branch compute
            wg0_sb = wgpool.tile([128, KW, 2 * GL], BF16)
            nc.gpsimd.dma_start(out=wg0_sb, in_=wg0b.ap())
            wg1_sb = wgpool.tile([128, KW, GL], BF16)
            nc.gpsimd.dma_start(out=wg1_sb, in_=wg1b.ap())
            xw_sb = wgpool.tile([128, KW, Wt], BF16)
            nc.gpsimd.dma_start(out=xw_sb, in_=xwt.ap())
            identb = wgpool.tile([128, 128], BF16)
            make_identity(nc, identb)

            og_sb = ogpool.tile([2 * L, GPC], F32)
            xtq_r = xtq.ap()

            for j in range(GPC):
                if j == 2:
                    # emit whole branch mid-stream: overlaps groups 2..7
                    emit_whole(nc, wgpool, wtpool, pw, pt,
                               wg0_sb, wg1_sb, xw_sb, identb)
                c0 = j * TB
                xt_sb = xtpool.tile([128, IN // 128, TB], FP8)
                nc.sync.dma_start(out=xt_sb,
                                  in_=xtq_r[j * 128:(j + 1) * 128])
                pf_sb = pfall_sb[0:1, c0:c0 + TB]

                # L1: h1 = relu(Wl0.T @ xt + bl0)   [512, TB] fp8
                h1_sb = h1pool.tile([128, GL // 128, TB], FP8)
                for mc in range(GL // 128):
                    for s in range(NCH):
                        n0 = s * NSZ
                        ps = ph1.tile([128, NSZ], F32, tag="ps")
                        for k2 in range(IN // 256):
                            nc.tensor.matmul(
                                ps,
                                wl0_sb[:, 2 * k2:2 * k2 + 2,
                                       mc * 128:(mc + 1) * 128],
                                xt_sb[:, 2 * k2:2 * k2 + 2, n0:n0 + NSZ],
                                start=(k2 == 0), stop=(k2 == IN // 256 - 1),
                                perf_mode=DR)
                        nc.scalar.activation(h1_sb[:, mc, n0:n0 + NSZ], ps,
                                             ACTF.Relu,
                                             bias=bl0_sb[:, mc:mc + 1])

                # L2: xt2 = relu(Wl1.T @ h1 + bl1)  [256, TB] bf16
                xt2_sb = xt2pool.tile([128, LC // 128, TB], BF16)
                for mc in range(LC // 128):
                    for s in range(NCH):
                        n0 = s * NSZ
                        ps = ph1.tile([128, NSZ], F32, tag="ps")
                        for k2 in range(GL // 256):
                            nc.tensor.matmul(
                                ps,
                                wl1_sb[:, 2 * k2:2 * k2 + 2,
                                       mc * 128:(mc + 1) * 128],
                                h1_sb[:, 2 * k2:2 * k2 + 2, n0:n0 + NSZ],
                                start=(k2 == 0), stop=(k2 == GL // 256 - 1),
                                perf_mode=DR)
                        evac(xt2_sb[:, mc, n0:n0 + NSZ], ps,
                             bl1_sb[:, mc:mc + 1])

                # scores(dup)+u in ONE [32, NSZ] PSUM group; engine APs
                # must start at the memref base, so u rows move to a
                # partition-base-0 tile via SBUF->SBUF DMA (DMA has no
                # partition-alignment restriction).
                su_sb = supool.tile([NSU, TB], F32, tag="su")
                u_sb = supool.tile([2 * L, TB], F32, tag="u")
                for s in range(NCH):
                    n0 = s * NSZ
                    ps = psc.tile([NSU, NSZ], F32, tag="psc")
                    for kt in range(LC // 128):
                        nc.tensor.matmul(
                            ps, scw_sb[:, kt, :],
                            xt2_sb[:, kt, n0:n0 + NSZ],
                            start=(kt == 0), stop=False)
                    nc.tensor.matmul(ps, negbig_sb[0:1, :],
                                     pf_sb[0:1, n0:n0 + NSZ],
                                     start=False, stop=True)
                    nc.vector.tensor_copy(su_sb[:, n0:n0 + NSZ], ps)
                nc.scalar.dma_start(out=u_sb, in_=su_sb[2 * L:NSU, :])
                sc_sb = su_sb[0:2 * L, :]

                # segment softmax + fused u-reduction.  Scores are O(1)
                # (bounded inputs), so exp() needs no max-shift: softmax is
                # shift-invariant and fp32 exp cannot overflow here.
                ex_sb = supool.tile([2 * L, TB], F32, tag="ex")
                den = smpool.tile([2 * L, 1], F32, tag="den")
                nc.scalar.activation(ex_sb, sc_sb, ACTF.Exp,
                                     bias=0.0, accum_out=den)
                prod_sb = supool.tile([2 * L, TB], F32, tag="prod")
                red = smpool.tile([2 * L, 1], F32, tag="red")
                nc.vector.scalar_tensor_tensor(
                    prod_sb, ex_sb, 1.0, u_sb,
                    op0=ALU.mult, op1=ALU.mult, accum_out=red)
                rden = smpool.tile([2 * L, 1], F32, tag="rden")
                nc.vector.reciprocal(rden, den)
                nc.vector.tensor_scalar_mul(og_sb[:, j:j + 1], red, rden)

            nc.sync.dma_start(out=out16.ap(), in_=og_sb)

    nc.compile()
    return nc


def _get_program(key):
    if key not in _prog_cache:
        _prog_cache[key] = _build_program(*key)
    return _prog_cache[key]


def kernel(**inputs):
    x = np.ascontiguousarray(np.asarray(inputs["x"], dtype=np.float32))
    group = np.asarray(inputs["group"]).astype(np.int64)
    itype = np.asarray(inputs["instance_type"]).astype(np.int64)
    Wl0 = np.asarray(inputs["Wl0"], np.float32)
    bl0 = np.asarray(inputs["bl0"], np.float32)
    Wl1 = np.asarray(inputs["Wl1"], np.float32)
    bl1 = np.asarray(inputs["bl1"], np.float32)
    Wg0 = np.asarray(inputs["Wg0"], np.float32)
    bg0 = np.asarray(inputs["bg0"], np.float32)
    Wg1 = np.asarray(inputs["Wg1"], np.float32)
    bg1 = np.asarray(inputs["bg1"], np.float32)
    Wk = np.asarray(inputs["Wk"], np.float32)
    bk = np.asarray(inputs["bk"], np.float32)
    Wv = np.asarray(inputs["Wv"], np.float32)
    bv = np.asarray(inputs["bv"], np.float32)
    latent = np.asarray(inputs["latent"], np.float32)
    Wout = np.asarray(inputs["Wout"], np.float32)
    bout = np.asarray(inputs["bout"], np.float32)

    # ---- host bucketing ----
    is_tile = itype == 1
    is_whole = itype == 0
    tile_idx = [np.where(is_tile & (group == g))[0] for g in range(G)]
    whole_idx = [np.where(is_whole & (group == g))[0] for g in range(G)]
    ng = np.array([len(ix) for ix in tile_idx])
    wg = np.array([len(ix) for ix in whole_idx])
    TB = max(384, _ceil_to(int(ng.max()), 384))
    WB = max(1, int(wg.max()))
    T = GPC * TB
    Wt = GPC * WB
    KW = IN // 128 + 1

    # ---- shared staged weights ----
    scale = 1.0 / math.sqrt(LC)
    wkl = (Wk @ latent.T) * scale                     # [LC, L]
    wvo = np.stack([Wv @ Wout[GL + l * LC:GL + (l + 1) * LC, c]
                    for l in range(L) for c in range(NCLS)], axis=1)  # [LC,16]
    scw_host = np.concatenate(
        [np.repeat(wkl, NCLS, axis=1), wvo], axis=1)  # [LC, 32]
    negbig_host = np.zeros((1, NSU), np.float32)
    negbig_host[0, :2 * L] = NEGBIG

    wg0b = np.zeros((KW * 128, 2 * GL), np.float32)
    wg0b[:IN] = Wg0
    wg0b[IN] = bg0
    wg1b = np.zeros((KW * 128, GL), np.float32)
    wg1b[:IN] = Wg1
    wg1b[IN] = bg1

    def tile_km(a, np_dt):
        """[K, M] -> [128, K//128, M] (SBUF partition-major tiling)."""
        k, m = a.shape
        return np.ascontiguousarray(
            np.asarray(a, np.float32).astype(np_dt)
            .reshape(k // 128, 128, m).transpose(1, 0, 2))

    shared = dict(
        wl0q=tile_km(Wl0, NP_FP8), wl1q=tile_km(Wl1, NP_FP8),
        scw=tile_km(scw_host, NP_BF16),
        negbig=negbig_host.astype(NP_BF16),
        bl0t=np.ascontiguousarray(bl0.reshape(-1, 128).T),
        bl1t=np.ascontiguousarray(bl1.reshape(-1, 128).T),
        wg0b=tile_km(wg0b, NP_BF16), wg1b=tile_km(wg1b, NP_BF16),
    )

    xq_all = x.astype(NP_FP8)
    xb_all = x.astype(NP_BF16)

    in_maps = []
    for c in range(N_CORES):
        xtq = np.zeros((GPC * 128, IN // 128, TB), NP_FP8)
        xwt = np.zeros((KW * 128, Wt), np.float32)
        xwt[IN] = 1.0
        padf = np.ones((1, T), NP_BF16)
        for j in range(GPC):
            g = c * GPC + j
            ti, wi = tile_idx[g], whole_idx[g]
            n = len(ti)
            xtq[j * 128:(j + 1) * 128, :, :n] = (
                xq_all[ti].T.reshape(IN // 128, 128, n).transpose(1, 0, 2))
            xwt[:IN, j * WB:j * WB + len(wi)] = x[wi].T
            padf[0, j * TB:j * TB + n] = 0
        in_maps.append(dict(xtq=xtq, xwt=tile_km(xwt, NP_BF16), padf=padf,
                            **shared))

    nc = _get_program((TB, WB))
    trace = os.environ.get("KERNEL_TRACE") == "1"
    if trace:
        _install_ntff_shim()
    res = run_bass_kernel_spmd(nc, in_maps, core_ids=list(range(N_CORES)),
                               trace=trace)
    global last_exec_time_ns, last_mean_exec_time_ns
    last_exec_time_ns = res.exec_time_ns
    last_mean_exec_time_ns = res.mean_exec_time_ns

    # ---- host assembly ----
    whole_agg = np.empty((G, GL), np.float32)
    t2 = np.empty((G, NCLS), np.float32)
    for c in range(N_CORES):
        ow = res.results[c]["out_w"]          # [Wt, GL]
        og = res.results[c]["out16"]          # [16, GPC]
        for j in range(GPC):
            g = c * GPC + j
            if wg[g] == 0:
                whole_agg[g] = -np.inf
            else:
                whole_agg[g] = ow[j * WB:j * WB + wg[g]].max(axis=0)
            t2[g] = og[:, j].reshape(L, NCLS).sum(axis=0)
    const = np.array([sum(bv @ Wout[GL + l * LC:GL + (l + 1) * LC, c]
                          for l in range(L)) for c in range(NCLS)], np.float32)
    return (whole_agg @ Wout[:GL] + t2 + const + bout).astype(np.float32)


# revision 26
# speedup vs baseline: 1.3223x; 1.0170x over previous
"""Trainium2 Bass kernel for nn_MILPFAttnTrexModel (segment_reduce).

Contract: kernel(**inputs) takes the FULL unsharded inputs (numpy arrays, keys
as in reference.setup_inputs()) and returns the FULL [G, NC] float32 output.

Strategy (8 NeuronCores, SPMD — one program, per-core data):
  - Host buckets rows by group; 8 groups per core, each group's tile-instance
    rows padded to a uniform block of TB columns. Tile-instance inputs are
    shipped feature-major AND pre-quantized to fp8 e4m3; the 2-layer MLP runs
    as fp8 DoubleRow matmuls (K=256 per pass, 0.5 cycles/row).
  - Algebraic fold: the whole v-projection + out_group matmul collapse into
    16 extra bf16 "u" columns next to the scores:
        out[g,l,c] = sum_i attn[i,l] * (v[i] . Wout[GL+l*LC:+LC, c])
                   = sum_i attn[i,l] * (xt2[i] . (Wv @ Wout_lc)) + bv.Wout_lc
    Scores columns are duplicated per class so ex[16,TB] and u[16,TB] align
    partition-wise; one fused DVE multiply+row-reduce gives the result.
    bk and bv contributions are softmax-invariant / constant and folded on
    host.  Pad columns are killed exactly by an extra K-row in the scores
    matmul contributing -1e30 * pad_flag.
  - Whole-image branch (1 instance per group): transposed bf16 MLP with the
    bias folded in as an extra contraction row; [Wt, GL] output, segment max
    on host.
"""

import math
import os
import numpy as np
import ml_dtypes

import concourse.bacc as bacc
import concourse.tile as tile
from concourse import mybir
from concourse.bass_utils import run_bass_kernel_spmd
from concourse.masks import make_identity

# Set by the most recent kernel() call when KERNEL_TRACE=1 (dev-only).
last_exec_time_ns = None
last_mean_exec_time_ns = None


def _install_ntff_shim():
    """Register the axon NTFF profile hook if the image's antenv lacks it."""
    import sys, types
    try:
        import antenv.axon_hooks  # noqa: F401
        return
    except ImportError:
        pass
    m = types.ModuleType("antenv.axon_hooks")
    m._hook = None
    m.set_axon_ntff_profile_hook = lambda h: setattr(m, "_hook", h)
    m.get_axon_ntff_profile_hook = lambda: m._hook
    sys.modules["antenv.axon_hooks"] = m
    import antenv
    antenv.axon_hooks = m
    from trn_agent_boot.trn_boot import _ntff_profile_via_ctypes
    m.set_axon_ntff_profile_hook(
        _ntff_profile_via_ctypes("/opt/axon/libaxon_pjrt.so"))

F32 = mybir.dt.float32
BF16 = mybir.dt.bfloat16
FP8 = mybir.dt.float8e4
AX = mybir.AxisListType
ALU = mybir.AluOpType
ACTF = mybir.ActivationFunctionType
DR = mybir.MatmulPerfMode.DoubleRow

NP_FP8 = ml_dtypes.float8_e4m3
NP_BF16 = ml_dtypes.bfloat16

N_CORES = 8
G = 64
GPC = G // N_CORES          # groups per core
IN = 1024
GL = 512
LC = 256
L = 8
NCLS = 2
NSU = 2 * L * NCLS          # 32 = 16 dup-score cols + 16 u cols
NEGBIG = -1.0e30

_prog_cache = {}


def _ceil_to(x, m):
    return ((x + m - 1) // m) * m


def _build_program(TB, WB):
    """Build the SPMD Tile program for block sizes (TB, WB)."""
    T = GPC * TB
    Wt = GPC * WB
    NSZ = 384
    NCH = TB // NSZ          # N-chunks per group
    KW = IN // 128 + 1       # whole-branch K chunks incl. bias-aug row

    nc = bacc.Bacc("TRN2", target_bir_lowering=False, debug=False,
                   num_devices=N_CORES)

    xtq = nc.dram_tensor("xtq", [GPC * 128, IN // 128, TB], FP8,
                         kind="ExternalInput")
    padf = nc.dram_tensor("padf", [1, T], BF16, kind="ExternalInput")
    wl0q = nc.dram_tensor("wl0q", [128, IN // 128, GL], FP8,
                          kind="ExternalInput")
    wl1q = nc.dram_tensor("wl1q", [128, GL // 128, LC], FP8,
                          kind="ExternalInput")
    scw = nc.dram_tensor("scw", [128, LC // 128, NSU], BF16,
                         kind="ExternalInput")
    bl0t = nc.dram_tensor("bl0t", [128, GL // 128], F32, kind="ExternalInput")
    bl1t = nc.dram_tensor("bl1t", [128, LC // 128], F32, kind="ExternalInput")
    negbig_in = nc.dram_tensor("negbig", [1, NSU], BF16, kind="ExternalInput")
    xwt = nc.dram_tensor("xwt", [128, KW, Wt], BF16, kind="ExternalInput")
    wg0b = nc.dram_tensor("wg0b", [128, KW, 2 * GL], BF16,
                          kind="ExternalInput")
    wg1b = nc.dram_tensor("wg1b", [128, KW, GL], BF16, kind="ExternalInput")
    out16 = nc.dram_tensor("out16", [2 * L * NCLS // 2, GPC], F32,
                           kind="ExternalOutput")
    out_w = nc.dram_tensor("out_w", [Wt, GL], F32, kind="ExternalOutput")

    tick = [0]

    def evac(out_ap, in_ap, bias_ap=None, force=None):
        """PSUM -> SBUF eviction, optionally fused bias-add + relu."""
        use_dve = (tick[0] % 2 == 0) if force is None else (force == "dve")
        tick[0] += 1
        if bias_ap is None:
            if use_dve:
                nc.vector.tensor_copy(out_ap, in_ap)
            else:
                nc.scalar.copy(out_ap, in_ap)
        else:
            if use_dve:
                nc.vector.tensor_scalar(out_ap, in_ap, bias_ap, 0.0,
                                        op0=ALU.add, op1=ALU.max)
            else:
                nc.scalar.activation(out_ap, in_ap, ACTF.Relu, bias=bias_ap)

    def emit_whole(nc, wgpool, wtpool, pw, pt, wg0_sb, wg1_sb, xw_sb, identb):
        # L1w: h1wT [Wt, 2GL] = relu(xw.T @ Wg0 + bg0)  (bias via aug K-row)
        h1w_sb = wtpool.tile([Wt, 2 * GL], BF16)
        for nchk in range(2 * GL // 512):
            ps = pw.tile([Wt, 512], F32, tag="pw")
            for kt in range(KW):
                nc.tensor.matmul(
                    ps, xw_sb[:, kt, :],
                    wg0_sb[:, kt, nchk * 512:(nchk + 1) * 512],
                    start=(kt == 0), stop=(kt == KW - 1))
            nc.scalar.activation(h1w_sb[:, nchk * 512:(nchk + 1) * 512],
                                 ps, ACTF.Relu)

        # transpose h1wT -> [128, KW, Wt] (+ bias-aug plane)
        h1t_sb = wtpool.tile([128, KW, Wt], BF16)
        nc.vector.memset(h1t_sb[:, KW - 1, :], 0.0)
        nc.vector.memset(h1t_sb[0:1, KW - 1, :], 1.0)
        for c in range(2 * GL // 128):
            pst = pt.tile([128, Wt], BF16, tag="pt")
            nc.tensor.transpose(pst, h1w_sb[:, c * 128:(c + 1) * 128],
                                identb[0:Wt, 0:Wt])
            nc.scalar.copy(h1t_sb[:, c, :], pst)

        # L2w: h2wT [Wt, GL] = relu(h1w.T @ Wg1 + bg1) -> host max
        h2w_sb = wtpool.tile([Wt, GL], F32)
        ps = pw.tile([Wt, 512], F32, tag="pw")
        for kt in range(KW):
            nc.tensor.matmul(ps, h1t_sb[:, kt, :], wg1_sb[:, kt, :],
                             start=(kt == 0), stop=(kt == KW - 1))
        nc.scalar.activation(h2w_sb, ps, ACTF.Relu)
        nc.gpsimd.dma_start(out=out_w.ap(), in_=h2w_sb)

    with tile.TileContext(nc) as tc:
        with (
            tc.tile_pool(name="weights", bufs=1) as wpool,
            tc.tile_pool(name="xt", bufs=2) as xtpool,
            tc.tile_pool(name="h1", bufs=2) as h1pool,
            tc.tile_pool(name="xt2", bufs=2) as xt2pool,
            tc.tile_pool(name="su", bufs=2) as supool,
            tc.tile_pool(name="small", bufs=2) as smpool,
            tc.tile_pool(name="ogall", bufs=1) as ogpool,
            tc.tile_pool(name="wg", bufs=1) as wgpool,
            tc.tile_pool(name="wtile", bufs=1) as wtpool,
            tc.tile_pool(name="ph1", bufs=4, space="PSUM") as ph1,
            tc.tile_pool(name="psc", bufs=2, space="PSUM") as psc,
            tc.tile_pool(name="pw", bufs=1, space="PSUM") as pw,
            tc.tile_pool(name="pt", bufs=1, space="PSUM") as pt,
        ):
            wl0_sb = wpool.tile([128, IN // 128, GL], FP8)
            nc.gpsimd.dma_start(out=wl0_sb, in_=wl0q.ap())
            bl0_sb = wpool.tile([128, GL // 128], F32)
            nc.gpsimd.dma_start(out=bl0_sb, in_=bl0t.ap())
            wl1_sb = wpool.tile([128, GL // 128, LC], FP8)
            nc.gpsimd.dma_start(out=wl1_sb, in_=wl1q.ap())
            bl1_sb = wpool.tile([128, LC // 128], F32)
            nc.gpsimd.dma_start(out=bl1_sb, in_=bl1t.ap())
            scw_sb = wpool.tile([128, LC // 128, NSU], BF16)
            nc.gpsimd.dma_start(out=scw_sb, in_=scw.ap())
            negbig_sb = wpool.tile([1, NSU], BF16)
            nc.gpsimd.dma_start(out=negbig_sb, in_=negbig_in.ap())
            pfall_sb = wpool.tile([1, T], BF16)
            nc.scalar.dma_start(out=pfall_sb, in_=padf.ap())

            # whole-branch loads start immediately on the gpsimd queue and
            # overlap tile-branch compute
            wg0_sb = wgpool.tile([128, KW, 2 * GL], BF16)
            nc.gpsimd.dma_start(out=wg0_sb, in_=wg0b.ap())
            wg1_sb = wgpool.tile([128, KW, GL], BF16)
            nc.gpsimd.dma_start(out=wg1_sb, in_=wg1b.ap())
            xw_sb = wgpool.tile([128, KW, Wt], BF16)
            nc.gpsimd.dma_start(out=xw_sb, in_=xwt.ap())
            identb = wgpool.tile([128, 128], BF16)
            make_identity(nc, identb)

            og_sb = ogpool.tile([2 * L, GPC], F32)
            xtq_r = xtq.ap()

            for j in range(GPC):
                if j == 2:
                    # emit whole branch mid-stream: overlaps groups 2..7
                    emit_whole(nc, wgpool, wtpool, pw, pt,
                               wg0_sb, wg1_sb, xw_sb, identb)
                c0 = j * TB
                xt_sb = xtpool.tile([128, IN // 128, TB], FP8)
                if j == 0:
                    # L1 k2=0,1 can start after the first 590KB half-load
                    nc.sync.dma_start(out=xt_sb[:, 0:4, :],
                                      in_=xtq_r[0:128, 0:4])
                    nc.sync.dma_start(out=xt_sb[:, 4:8, :],
                                      in_=xtq_r[0:128, 4:8])
                else:
                    nc.sync.dma_start(out=xt_sb,
                                      in_=xtq_r[j * 128:(j + 1) * 128])
                pf_sb = pfall_sb[0:1, c0:c0 + TB]

                # L1: h1 = relu(Wl0.T @ xt + bl0)   [512, TB] fp8
                h1_sb = h1pool.tile([128, GL // 128, TB], FP8)
                for mc in range(GL // 128):
                    for s in range(NCH):
                        n0 = s * NSZ
                        ps = ph1.tile([128, NSZ], F32, tag="ps")
                        for k2 in range(IN // 256):
                            nc.tensor.matmul(
                                ps,
                                wl0_sb[:, 2 * k2:2 * k2 + 2,
                                       mc * 128:(mc + 1) * 128],
                                xt_sb[:, 2 * k2:2 * k2 + 2, n0:n0 + NSZ],
                                start=(k2 == 0), stop=(k2 == IN // 256 - 1),
                                perf_mode=DR)
                        nc.scalar.activation(h1_sb[:, mc, n0:n0 + NSZ], ps,
                                             ACTF.Relu,
                                             bias=bl0_sb[:, mc:mc + 1])

                # L2: xt2 = relu(Wl1.T @ h1 + bl1)  [256, TB] bf16
                xt2_sb = xt2pool.tile([128, LC // 128, TB], BF16)
                for mc in range(LC // 128):
                    for s in range(NCH):
                        n0 = s * NSZ
                        ps = ph1.tile([128, NSZ], F32, tag="ps")
                        for k2 in range(GL // 256):
                            nc.tensor.matmul(
                                ps,
                                wl1_sb[:, 2 * k2:2 * k2 + 2,
                                       mc * 128:(mc + 1) * 128],
                                h1_sb[:, 2 * k2:2 * k2 + 2, n0:n0 + NSZ],
                                start=(k2 == 0), stop=(k2 == GL // 256 - 1),
                                perf_mode=DR)
                        evac(xt2_sb[:, mc, n0:n0 + NSZ], ps,
                             bl1_sb[:, mc:mc + 1])

                # scores(dup)+u in ONE [32, NSZ] PSUM group; engine APs
                # must start at the memref base, so u rows move to a
                # partition-base-0 tile via SBUF->SBUF DMA (DMA has no
                # partition-alignment restriction).
                su_sb = supool.tile([NSU, TB], F32, tag="su")
                u_sb = supool.tile([2 * L, TB], F32, tag="u")
                for s in range(NCH):
                    n0 = s * NSZ
                    ps = psc.tile([NSU, NSZ], F32, tag="psc")
                    for kt in range(LC // 128):
                        nc.tensor.matmul(
                            ps, scw_sb[:, kt, :],
                            xt2_sb[:, kt, n0:n0 + NSZ],
                            start=(kt == 0), stop=False)
                    nc.tensor.matmul(ps, negbig_sb[0:1, :],
                                     pf_sb[0:1, n0:n0 + NSZ],
                                     start=False, stop=True)
                    nc.vector.tensor_copy(su_sb[:, n0:n0 + NSZ], ps)
                nc.scalar.dma_start(out=u_sb, in_=su_sb[2 * L:NSU, :])
                sc_sb = su_sb[0:2 * L, :]

                # segment softmax + fused u-reduction.  Scores are O(1)
                # (bounded inputs), so exp() needs no max-shift: softmax is
                # shift-invariant and fp32 exp cannot overflow here.
                ex_sb = supool.tile([2 * L, TB], F32, tag="ex")
                den = smpool.tile([2 * L, 1], F32, tag="den")
                nc.scalar.activation(ex_sb, sc_sb, ACTF.Exp,
                                     bias=0.0, accum_out=den)
                prod_sb = supool.tile([2 * L, TB], F32, tag="prod")
                red = smpool.tile([2 * L, 1], F32, tag="red")
                nc.vector.scalar_tensor_tensor(
                    prod_sb, ex_sb, 1.0, u_sb,
                    op0=ALU.mult, op1=ALU.mult, accum_out=red)
                rden = smpool.tile([2 * L, 1], F32, tag="rden")
                nc.vector.reciprocal(rden, den)
                nc.vector.tensor_scalar_mul(og_sb[:, j:j + 1], red, rden)

            nc.sync.dma_start(out=out16.ap(), in_=og_sb)

    nc.compile()
    return nc


def _get_program(key):
    if key not in _prog_cache:
        _prog_cache[key] = _build_program(*key)
    return _prog_cache[key]


def kernel(**inputs):
    x = np.ascontiguousarray(np.asarray(inputs["x"], dtype=np.float32))
    group = np.asarray(inputs["group"]).astype(np.int64)
    itype = np.asarray(inputs["instance_type"]).astype(np.int64)
    Wl0 = np.asarray(inputs["Wl0"], np.float32)
    bl0 = np.asarray(inputs["bl0"], np.float32)
    Wl1 = np.asarray(inputs["Wl1"], np.float32)
    bl1 = np.asarray(inputs["bl1"], np.float32)
    Wg0 = np.asarray(inputs["Wg0"], np.float32)
    bg0 = np.asarray(inputs["bg0"], np.float32)
    Wg1 = np.asarray(inputs["Wg1"], np.float32)
    bg1 = np.asarray(inputs["bg1"], np.float32)
    Wk = np.asarray(inputs["Wk"], np.float32)
    bk = np.asarray(inputs["bk"], np.float32)
    Wv = np.asarray(inputs["Wv"], np.float32)
    bv = np.asarray(inputs["bv"], np.float32)
    latent = np.asarray(inputs["latent"], np.float32)
    Wout = np.asarray(inputs["Wout"], np.float32)
    bout = np.asarray(inputs["bout"], np.float32)

    # ---- host bucketing ----
    is_tile = itype == 1
    is_whole = itype == 0
    tile_idx = [np.where(is_tile & (group == g))[0] for g in range(G)]
    whole_idx = [np.where(is_whole & (group == g))[0] for g in range(G)]
    ng = np.array([len(ix) for ix in tile_idx])
    wg = np.array([len(ix) for ix in whole_idx])
    TB = max(384, _ceil_to(int(ng.max()), 384))
    WB = max(1, int(wg.max()))
    T = GPC * TB
    Wt = GPC * WB
    KW = IN // 128 + 1

    # ---- shared staged weights ----
    scale = 1.0 / math.sqrt(LC)
    wkl = (Wk @ latent.T) * scale                     # [LC, L]
    wvo = np.stack([Wv @ Wout[GL + l * LC:GL + (l + 1) * LC, c]
                    for l in range(L) for c in range(NCLS)], axis=1)  # [LC,16]
    scw_host = np.concatenate(
        [np.repeat(wkl, NCLS, axis=1), wvo], axis=1)  # [LC, 32]
    negbig_host = np.zeros((1, NSU), np.float32)
    negbig_host[0, :2 * L] = NEGBIG

    wg0b = np.zeros((KW * 128, 2 * GL), np.float32)
    wg0b[:IN] = Wg0
    wg0b[IN] = bg0
    wg1b = np.zeros((KW * 128, GL), np.float32)
    wg1b[:IN] = Wg1
    wg1b[IN] = bg1

    def tile_km(a, np_dt):
        """[K, M] -> [128, K//128, M] (SBUF partition-major tiling)."""
        k, m = a.shape
        return np.ascontiguousarray(
            np.asarray(a, np.float32).astype(np_dt)
            .reshape(k // 128, 128, m).transpose(1, 0, 2))

    shared = dict(
        wl0q=tile_km(Wl0, NP_FP8), wl1q=tile_km(Wl1, NP_FP8),
        scw=tile_km(scw_host, NP_BF16),
        negbig=negbig_host.astype(NP_BF16),
        bl0t=np.ascontiguousarray(bl0.reshape(-1, 128).T),
        bl1t=np.ascontiguousarray(bl1.reshape(-1, 128).T),
        wg0b=tile_km(wg0b, NP_BF16), wg1b=tile_km(wg1b, NP_BF16),
    )

    xq_all = x.astype(NP_FP8)
    xb_all = x.astype(NP_BF16)

    in_maps = []
    for c in range(N_CORES):
        xtq = np.zeros((GPC * 128, IN // 128, TB), NP_FP8)
        xwt = np.zeros((KW * 128, Wt), np.float32)
        xwt[IN] = 1.0
        padf = np.ones((1, T), NP_BF16)
        for j in range(GPC):
            g = c * GPC + j
            ti, wi = tile_idx[g], whole_idx[g]
            n = len(ti)
            xtq[j * 128:(j + 1) * 128, :, :n] = (
                xq_all[ti].T.reshape(IN // 128, 128, n).transpose(1, 0, 2))
            xwt[:IN, j * WB:j * WB + len(wi)] = x[wi].T
            padf[0, j * TB:j * TB + n] = 0
        in_maps.append(dict(xtq=xtq, xwt=tile_km(xwt, NP_BF16), padf=padf,
                            **shared))

    nc = _get_program((TB, WB))
    trace = os.environ.get("KERNEL_TRACE") == "1"
    if trace:
        _install_ntff_shim()
    res = run_bass_kernel_spmd(nc, in_maps, core_ids=list(range(N_CORES)),
                               trace=trace)
    global last_exec_time_ns, last_mean_exec_time_ns
    last_exec_time_ns = res.exec_time_ns
    last_mean_exec_time_ns = res.mean_exec_time_ns

    # ---- host assembly ----
    whole_agg = np.empty((G, GL), np.float32)
    t2 = np.empty((G, NCLS), np.float32)
    for c in range(N_CORES):
        ow = res.results[c]["out_w"]          # [Wt, GL]
        og = res.results[c]["out16"]          # [16, GPC]
        for j in range(GPC):
            g = c * GPC + j
            if wg[g] == 0:
                whole_agg[g] = -np.inf
            else:
                whole_agg[g] = ow[j * WB:j * WB + wg[g]].max(axis=0)
            t2[g] = og[:, j].reshape(L, NCLS).sum(axis=0)
    const = np.array([sum(bv @ Wout[GL + l * LC:GL + (l + 1) * LC, c]
                          for l in range(L)) for c in range(NCLS)], np.float32)
    return (whole_agg @ Wout[:GL] + t2 + const + bout).astype(np.float32)
